# revision 14
# baseline (speedup 1.0000x reference)
"""DifferentialAttention on 8 TRN2 NeuronCores.

Sharding: tensor-parallel over heads (2 heads per core), host sums the
8 partial output projections (not counted in HW exec time).

v4 (from v3 @ ~241us, v2 baseline @ ~247-350us):
- PE-bound kernel; the scalar-engine exp chain (~1us per score pair) is
  the attention serializer and PE idle pockets re-engage the HAM clock
  throttle (1.2 vs 2.4 GHz).  Design rules: keep the PE continuously
  busy, spread the 80 exp pairs evenly across the whole kernel.
- unit pipeline: for the 8 (head, chunk) attention units, scores of
  unit u+1 are emitted interleaved into the PV of unit u; qkv/proj
  matmul quanta fill all remaining slack (adaptive pacing).
- paired 2-bank PSUM tiles [128,2,512]: scores e1/e2 in one tile, ONE
  exp ACT per pair; row-tiled (64x128) score matmuls run concurrently.
- PV a1/a2 accumulate into ONE psum bank (cols 0:129 / 256:385).
- LN stats fused into the PV combine via accum_out; normalize on DVE.
- host relayouts x/wqk/wv so each chunk's activations arrive in 1-4
  large DMAs instead of 16 (the sync queue serializes issues at
  ~600ns each); warm-up matmuls cover the initial DMA window.
- PSUM: pe2 pool 3x2 banks + pa pool 2x1 banks = 8.
"""

import numpy as np

HEAD_DIM = 64
N_HEADS = 16
D_MODEL = 2048
SEQ = 2048
LAYER_IDX = 12
LN_EPS = 1e-5
N_CORES = 8
HPC = N_HEADS // N_CORES          # heads per core = 2
CHUNK = 512                       # query chunk width
NCHUNK = SEQ // CHUNK             # 4
NDT = D_MODEL // 128              # 16 d-tiles
NST = SEQ // 128                  # 16 s-tiles

_SYNC_CNT = [0]


def _patch_tile_drain(tile_mod, bass_rust):
    """The walrus build in this container encodes at most one sem wait per
    instruction; TileContext's exit drain carries one wait per producer
    proc. Split the extras onto single-wait NOPs."""
    from concourse.vector_clock import ScopedClock

    def patched(self, tick_clock, wait_clock):
        nc = self.nc
        drain_inst = nc.sync.drain()
        wait_clock.add_sem_waits(
            drain_inst.ins, ScopedClock({None: tick_clock.global_clock})
        )
        si = drain_inst.ins.sync_info
        waits = list(si.on_wait or [])
        if len(waits) > 1:
            si.on_wait = [waits[0]]
            for w in waits[1:]:
                nop = nc.sync.nop()
                nop.ins.sync_info = bass_rust.SyncInfo(on_wait=[w], on_update=[])
        nc.all_engine_barrier()
        popped = nc._tile_sem_poison_stack.pop()
        assert popped is self._sem_poison
        nc.clear_and_free_semaphores(list(self.sems.allocated().values()))
        nc.all_engine_barrier()

    tile_mod.TileContext._drain_and_barrier = patched


def _fix_sync_limits(nc, mybir, bass_rust):
    """Split multi-wait / multi-update instructions into single-wait NOP
    chains on the same engine queue (walrus single-sync-slot limit)."""

    def nop(engine, wait=None, update=None):
        _SYNC_CNT[0] += 1
        n = mybir.InstNoOp(name=f"syncsplit-{_SYNC_CNT[0]}", ins=[], outs=[])
        n.engine = engine
        n.sync_info = bass_rust.SyncInfo(
            on_wait=[wait] if wait is not None else [],
            on_update=[update] if update is not None else [],
        )
        return n

    for f in nc.m.functions:
        for b in f.blocks:
            out = []
            for inst in b.instructions:
                si = inst.sync_info
                post = []
                if si is not None:
                    waits = list(si.on_wait or [])
                    if len(waits) > 1:
                        for w in waits[:-1]:
                            out.append(nop(inst.engine, wait=w))
                        si.on_wait = [waits[-1]]
                    ups = list(si.on_update or [])
                    if len(ups) > 1:
                        si.on_update = [ups[0]]
                        for u in ups[1:]:
                            post.append(nop(inst.engine, update=u))
                out.append(inst)
                out.extend(post)
            b.instructions = out


def _install_ntff_shim():
    """Register the axon NTFF profile hook (used only when tracing)."""
    import sys, types
    if "antenv.axon_hooks" in sys.modules:
        return
    try:
        mod = types.ModuleType("antenv.axon_hooks")
        mod._hook = None
        mod.set_axon_ntff_profile_hook = lambda h: setattr(mod, "_hook", h)
        mod.get_axon_ntff_profile_hook = lambda: mod._hook
        sys.modules["antenv.axon_hooks"] = mod
        import antenv
        antenv.axon_hooks = mod
        from trn_agent_boot.trn_boot import _ntff_profile_via_ctypes
        mod.set_axon_ntff_profile_hook(
            _ntff_profile_via_ctypes("/opt/axon/libaxon_pjrt.so")
        )
    except Exception:
        pass


def _build_nc():
    import os
    WARM_N = int(os.environ.get("WARM_N", "16"))
    FILLJ = int(os.environ.get("FILLJ", "1"))    # fill quanta per pv j-block
    FILLCAP = int(os.environ.get("FILLCAP", "6"))
    SPLIT_DMA = bool(int(os.environ.get("SPLIT_DMA", "1")))
    import bass_rust
    import concourse.bass as bass
    import concourse.tile as tile
    import concourse.tile_sem_assignment as _tsa
    from concourse import mybir

    _patch_tile_drain(tile, bass_rust)
    # The Pool-engine proc sem plus 8 HWDGE sems overflows the sem range
    # this walrus build can encode in sem_clear; 7 DMA queues suffice.
    _tsa.NUM_HWDGE_SEMS = 7

    f32 = mybir.dt.float32
    bf16 = mybir.dt.bfloat16
    AT = mybir.ActivationFunctionType
    OP = mybir.AluOpType

    nc = bass.Bass()

    # host-relayouted: xT[c, p, d, col] = x[512c+col, 128d+p]
    xT = nc.dram_tensor("xT", [NCHUNK, 128, NDT, CHUNK], bf16,
                        kind="ExternalInput")
    wqkT = nc.dram_tensor("wqkT", [128, NDT, 512], bf16, kind="ExternalInput")
    wvT = nc.dram_tensor("wvT", [128, NDT, HPC * 128], bf16,
                         kind="ExternalInput")
    woT = nc.dram_tensor("woT", [HPC * 128, D_MODEL], bf16,
                         kind="ExternalInput")
    lamnegbc = nc.dram_tensor("lamnegbc", [128, HPC * 128], f32,
                              kind="ExternalInput")
    tri2 = nc.dram_tensor("tri2", [128, 256], bf16, kind="ExternalInput")
    y = nc.dram_tensor("y", [SEQ, D_MODEL], bf16, kind="ExternalOutput")

    SQEPS = float(np.sqrt(LN_EPS))

    with tile.TileContext(nc) as tc:
        import contextlib
        with contextlib.ExitStack() as ctx:
            consts = ctx.enter_context(tc.tile_pool(name="consts", bufs=1))
            main = ctx.enter_context(tc.tile_pool(name="main", bufs=1))
            p1w = ctx.enter_context(tc.tile_pool(name="p1w", bufs=1))
            p1x = ctx.enter_context(tc.tile_pool(name="p1x", bufs=2))
            pe12 = ctx.enter_context(tc.tile_pool(name="pe12", bufs=40))
            pw = ctx.enter_context(tc.tile_pool(name="pw", bufs=2))
            paux = ctx.enter_context(tc.tile_pool(name="paux", bufs=3))
            pot = ctx.enter_context(tc.tile_pool(name="pot", bufs=3))
            ppo = ctx.enter_context(tc.tile_pool(name="ppo", bufs=4))
            pyr = ctx.enter_context(tc.tile_pool(name="pyr", bufs=2))
            psm = ctx.enter_context(tc.tile_pool(name="psm", bufs=12))
            # PSUM: 3*2 + 2*1 = 8 banks
            pe2 = ctx.enter_context(tc.tile_pool(name="pe2", bufs=3, space="PSUM"))
            pa = ctx.enter_context(tc.tile_pool(name="pa", bufs=2, space="PSUM"))

            # ---- constants ----
            lam_bc = consts.tile([128, HPC * 128], f32)
            tri_sb = consts.tile([128, 2, 128], bf16)
            warm = consts.tile([128, 1], f32)
            nc.vector.memset(warm[:], 0.0)
            nc.scalar.activation(warm[:], warm[:], AT.Exp)

            # ---- persistent activations ----
            # qk layout: [128 dims, {q0,q1,k0,k1}, SEQ]
            qk_sb = main.tile([128, 4, SEQ], bf16, name="qk")
            # v tile layout per 128-key block: [head][v(128) | 1 | v'(128) | 1]
            vb = main.tile([128, NST, HPC, 258], bf16, name="vb")
            wo_sb = [main.tile([128, SEQ], bf16, name=f"wo{i}") for i in range(HPC)]
            wqk_sb = p1w.tile([128, NDT, 512], bf16, name="wqk")
            wv_sb = p1w.tile([128, NDT, HPC * 128], bf16, name="wv")

            # =================== PE warm-up ===================
            # Garbage matmuls (uninitialized source tile, unread psum) keep
            # the PE busy while the first x tiles DMA in, so the HAM clock
            # gate opens before real work starts.
            wsrc = consts.tile([128, 512], bf16)
            nc.vector.memset(wsrc[:], 0.0)
            # ones columns of every v tile (cols 128, 257 per head)
            nc.vector.memset(vb[:, :, :, 128:129], 1.0)
            nc.vector.memset(vb[:, :, :, 257:258], 1.0)
            wp = pe2.tile([128, 2, CHUNK], f32, tag="pe2", name="warmmm")
            for _ in range(WARM_N):
                nc.tensor.matmul(wp[:, 0], wsrc[:, 0:128], wsrc[:],
                                 start=True, stop=True)

            # =================== phase-1 chunk 0 (d-outer) ===================
            def ph1_c0():
                # DMA priority by issue order: the DMA engines round-robin
                # packets across active queues, so late-needed transfers are
                # issued from the gpsimd queue, which reaches them later.
                # Sub-tile dependency tracking lets the d-group matmuls
                # start as soon as their own block has landed.
                # The DMA engines drain the hardware queues close to
                # serially, so within one queue issue order IS priority.
                # Interleave weight/x blocks in the order the d-outer
                # chain consumes them, all on the sync queue.
                xcb = p1x.tile([128, NDT, CHUNK], bf16, tag="xcb")
                for lo, hi in ((0, 2), (2, 4), (4, 8), (8, 12), (12, 16)):
                    bsl = slice(lo, hi)
                    nc.sync.dma_start(wqk_sb[:, bsl, :], wqkT[:, bsl, :])
                    nc.sync.dma_start(xcb[:, bsl, :], xT[0, :, bsl, :])
                nc.sync.dma_start(wv_sb[:], wvT[:])
                geng = nc.gpsimd if SPLIT_DMA else nc.sync
                geng.dma_start(lam_bc[:], lamnegbc[:])
                geng.dma_start(tri_sb[:], tri2[:])
                # q/k: d-outer across both pair accumulators
                qpA = pe2.tile([128, 2, CHUNK], f32, tag="pe2", name="qpA")
                qpB = pe2.tile([128, 2, CHUNK], f32, tag="pe2", name="qpB")
                for d in range(NDT):
                    st, sp = (d == 0), (d == NDT - 1)
                    nc.tensor.matmul(qpA[:, 0], wqk_sb[:, d, 0:128],
                                     xcb[:, d, :], start=st, stop=sp)
                    nc.tensor.matmul(qpA[:, 1], wqk_sb[:, d, 128:256],
                                     xcb[:, d, :], start=st, stop=sp)
                    nc.tensor.matmul(qpB[:, 0], wqk_sb[:, d, 256:384],
                                     xcb[:, d, :], start=st, stop=sp)
                    nc.tensor.matmul(qpB[:, 1], wqk_sb[:, d, 384:512],
                                     xcb[:, d, :], start=st, stop=sp)
                nc.vector.tensor_copy(qk_sb[:, 0:2, 0:CHUNK], qpA[:])
                nc.vector.tensor_copy(qk_sb[:, 2:4, 0:CHUNK], qpB[:])
                for ss in range(4):
                    vp = pe2.tile([128, 2, CHUNK], f32, tag="pe2", name="vp")
                    for d in range(NDT):
                        nc.tensor.matmul(
                            vp[:, 0, 0:256], xcb[:, d, 128 * ss:128 * (ss + 1)],
                            wv_sb[:, d, :], start=(d == 0), stop=(d == NDT - 1))
                    for hh in range(HPC):
                        hsl = slice(128 * hh, 128 * (hh + 1))
                        nc.vector.tensor_copy(
                            vb[:, ss, hh, 0:128], vp[:, 0, hsl])
                        nc.gpsimd.tensor_tensor(
                            vb[:, ss, hh, 129:257], vb[:, ss, hh, 0:128],
                            lam_bc[:, hsl], OP.mult)

            # =================== phase-1 chunks 1..3 (generators) ==========
            # qk and v are separate generators: scores of chunk c only need
            # qk(c) emitted, pv additionally needs v(c); splitting lets the
            # next chunk's score/exp chain start ~7us earlier.
            def ph1_qk_gen(c, xcb_box):
                csl = slice(CHUNK * c, CHUNK * (c + 1))
                xcb = p1x.tile([128, NDT, CHUNK], bf16, tag="xcb")
                xcb_box.append(xcb)
                xeng = nc.scalar if (SPLIT_DMA and c == 1) else nc.sync
                xeng.dma_start(xcb[:, 0:8, :], xT[c, :, 0:8, :])
                xeng.dma_start(xcb[:, 8:16, :], xT[c, :, 8:16, :])
                if c == 1:
                    for i in range(HPC):
                        nc.sync.dma_start(
                            wo_sb[i][:], woT[128 * i:128 * (i + 1), :])
                yield
                for pr in range(2):
                    qp = pe2.tile([128, 2, CHUNK], f32, tag="pe2", name="qp")
                    for d in range(NDT):
                        st, sp = (d == 0), (d == NDT - 1)
                        nc.tensor.matmul(
                            qp[:, 0], wqk_sb[:, d, 256 * pr:256 * pr + 128],
                            xcb[:, d, :], start=st, stop=sp)
                        nc.tensor.matmul(
                            qp[:, 1], wqk_sb[:, d, 256 * pr + 128:256 * pr + 256],
                            xcb[:, d, :], start=st, stop=sp)
                        yield
                    nc.vector.tensor_copy(qk_sb[:, 2 * pr:2 * pr + 2, csl], qp[:])

            def ph1_v_gen(c, xcb_box):
                xcb = xcb_box[0]
                for ss in range(4):
                    t = 4 * c + ss
                    vp = pe2.tile([128, 2, CHUNK], f32, tag="pe2", name="vp")
                    for d in range(NDT):
                        nc.tensor.matmul(
                            vp[:, 0, 0:256], xcb[:, d, 128 * ss:128 * (ss + 1)],
                            wv_sb[:, d, :], start=(d == 0), stop=(d == NDT - 1))
                        if d % 4 == 3:
                            yield
                    for hh in range(HPC):
                        hsl = slice(128 * hh, 128 * (hh + 1))
                        nc.vector.tensor_copy(
                            vb[:, t, hh, 0:128], vp[:, 0, hsl])
                        nc.gpsimd.tensor_tensor(
                            vb[:, t, hh, 129:257], vb[:, t, hh, 0:128],
                            lam_bc[:, hsl], OP.mult)
                yield

            # =================== scores (generator: one pair per quantum) ==
            def scores_gen(h, c, ets):
                for t in range(4 * (c + 1)):
                    diag = t >= 4 * c
                    f0 = 128 * (t - 4 * c) if diag else 0
                    sl = slice(f0, CHUNK)
                    qsl = slice(CHUNK * c + f0, CHUNK * (c + 1))
                    ep = pe2.tile([128, 2, CHUNK], f32, tag="pe2", name="ep")
                    nc.tensor.matmul(
                        ep[:, 0, sl], qk_sb[0:64, 2 + h, 128 * t:128 * (t + 1)],
                        qk_sb[0:64, h, qsl], start=True, stop=True)
                    nc.tensor.matmul(
                        ep[:, 1, sl], qk_sb[64:128, 2 + h, 128 * t:128 * (t + 1)],
                        qk_sb[64:128, h, qsl], start=True, stop=True)
                    et = pe12.tile([128, 2, CHUNK], bf16, tag="e12")
                    nc.scalar.activation(et[:, :, sl], ep[:, :, sl], AT.Exp)
                    if diag:
                        dsl = slice(f0, f0 + 128)
                        nc.vector.tensor_tensor(
                            et[:, :, dsl], et[:, :, dsl], tri_sb[:], OP.mult)
                    ets[t] = et
                    yield

            # =================== PV + LN (generator: one j per quantum) ====
            def pv_ln_gen(h, c, ets):
                w_t = pw.tile([128, 4, 128], f32, tag="w")
                s18 = psm.tile([128, 8], f32, tag="s18")
                epsd2 = psm.tile([128, 4], f32, tag="ed")
                for j in range(4):
                    nt = 4 * c + j + 1
                    jsl = slice(128 * j, 128 * (j + 1))
                    pvp = pa.tile([128, 512], f32, tag="pa", name="pvp")
                    for t in range(nt):
                        nc.tensor.matmul(
                            pvp[:, 0:129], ets[t][:, 0, jsl],
                            vb[:, t, h, 0:129],
                            start=(t == 0), stop=(t == nt - 1))
                    for t in range(nt):
                        nc.tensor.matmul(
                            pvp[:, 256:385], ets[t][:, 1, jsl],
                            vb[:, t, h, 129:258],
                            start=(t == 0), stop=(t == nt - 1))
                    # w = (d2/d1)*a1 + a2'   (= d2 * w_true, LN-scale-invariant)
                    rd1 = psm.tile([128, 1], f32, tag="rd1")
                    nc.vector.reciprocal(rd1[:], pvp[:, 128:129])
                    scol = psm.tile([128, 1], f32, tag="scol")
                    nc.vector.tensor_tensor(
                        scol[:], pvp[:, 384:385], rd1[:], OP.mult)
                    # DVE reads at most one PSUM operand per instruction:
                    # (d2/d1)*a1 -> sbuf, then + a2' (accumulating s1 for LN)
                    sa1 = paux.tile([128, 128], f32, tag="sa1")
                    nc.vector.tensor_scalar_mul(sa1[:], pvp[:, 0:128], scol[:])
                    nc.vector.scalar_tensor_tensor(
                        w_t[:, j], in0=sa1[:], scalar=1.0, in1=pvp[:, 256:384],
                        op0=OP.mult, op1=OP.add, accum_out=s18[:, j:j + 1])
                    nc.scalar.activation(
                        epsd2[:, j:j + 1], pvp[:, 384:385], AT.Square,
                        scale=SQEPS)
                    wsq = paux.tile([128, 128], f32, tag="wsq")
                    nc.vector.scalar_tensor_tensor(
                        wsq[:], in0=w_t[:, j], scalar=1.0, in1=w_t[:, j],
                        op0=OP.mult, op1=OP.mult,
                        accum_out=s18[:, 4 + j:5 + j])
                    yield
                # ---- LN stats (free-dim, per-partition) ----
                s1c = s18[:, 0:4]
                s2c = s18[:, 4:8]
                t0 = psm.tile([128, 4], f32, tag="t0")
                nc.vector.scalar_tensor_tensor(
                    t0[:], in0=s1c, scalar=1.0 / 128, in1=s1c,
                    op0=OP.mult, op1=OP.mult)
                t1 = psm.tile([128, 4], f32, tag="t1")
                nc.vector.tensor_tensor(t1[:], s2c, t0[:], OP.subtract)
                varep = psm.tile([128, 4], f32, tag="ve")
                nc.vector.scalar_tensor_tensor(
                    varep[:], in0=t1[:], scalar=1.0 / 128, in1=epsd2[:],
                    op0=OP.mult, op1=OP.add)
                lnv = psm.tile([128, 4], f32, tag="lnv")
                nc.scalar.activation(lnv[:], varep[:], AT.Ln)
                rstd = psm.tile([128, 4], f32, tag="rstd")
                nc.scalar.activation(rstd[:], lnv[:], AT.Exp, scale=-0.5)
                nmr = psm.tile([128, 4], f32, tag="nmr")
                nc.vector.scalar_tensor_tensor(
                    nmr[:], in0=s1c, scalar=1.0 / 128, in1=rstd[:],
                    op0=OP.mult, op1=OP.mult)
                outT_t = pot.tile([128, 4, 128], bf16, tag="outT")
                for j in range(4):
                    nc.vector.tensor_scalar(
                        outT_t[:, j], w_t[:, j], rstd[:, j:j + 1],
                        nmr[:, j:j + 1], op0=OP.mult, op1=OP.subtract)
                pv_ln_gen.out = outT_t

            # =================== transpose ===================
            # DMA-xbar transpose: zero PE cycles, and the pa pool is left
            # entirely to the PV accumulators.
            def tr(outT_t):
                po = ppo.tile([128, 4, 128], bf16, tag="po")
                for j in range(4):
                    nc.sync.dma_start_transpose(po[:, j], outT_t[:, j])
                return po

            # =================== projection (generator) ===================
            def proj_gen(c, po_pair):
                for st_l in range(4):
                    st = 4 * c + st_l
                    yr = pyr.tile([128, SEQ], bf16, tag="yr")
                    for pr in range(2):
                        yp = pe2.tile([128, 2, CHUNK], f32, tag="pe2",
                                      name="yp")
                        for half in range(2):
                            osl = slice(1024 * pr + 512 * half,
                                        1024 * pr + 512 * (half + 1))
                            for n, i in enumerate((0, 1)):
                                nc.tensor.matmul(
                                    yp[:, half], po_pair[i][:, st_l],
                                    wo_sb[i][:, osl],
                                    start=(n == 0), stop=(n == HPC - 1))
                        ysl = slice(1024 * pr, 1024 * (pr + 1))
                        if pr == 0:
                            nc.vector.tensor_copy(yr[:, ysl], yp[:])
                        else:
                            nc.scalar.copy(yr[:, ysl], yp[:])
                        yield
                    nc.sync.dma_start(y[128 * st:128 * (st + 1), :], yr[:])

            # =================== schedule ===================
            fillq = []
            state = {"est": 0, "pairs": 80}

            def add_fill(g, est):
                fillq.append(g)
                state["est"] += est

            # FIFO: exactly one filler generator is ever mid-flight, so at
            # most one long accumulation chain holds a pe2 buffer at a time
            # (two concurrent chains + two score pairs would exceed the 3
            # pe2 buffers and deadlock the in-order PE queue).
            def fill(n):
                while n > 0 and fillq:
                    try:
                        next(fillq[0])
                        state["est"] -= 1
                        n -= 1
                    except StopIteration:
                        fillq.pop(0)

            def fillp():
                # adaptive pacing: spread remaining filler quanta evenly
                # over the remaining score pairs
                state["pairs"] -= 1
                k = -(-state["est"] // max(state["pairs"], 1))
                fill(min(k, FILLCAP))

            def drain(g):
                while True:
                    try:
                        next(g)
                        state["est"] -= 1
                    except StopIteration:
                        break
                if g in fillq:
                    fillq.remove(g)

            def step(g):
                try:
                    next(g)
                    return True
                except StopIteration:
                    return False

            QK_EST = 1 + 2 * NDT      # 33
            V_EST = NDT + 1           # 17
            PROJ_EST = 8

            def make_ph1(c):
                box = []
                qkg = ph1_qk_gen(c, box)
                vg = ph1_v_gen(c, box)
                add_fill(qkg, QK_EST)
                add_fill(vg, V_EST)
                return (qkg, vg)

            ph1_c0()
            ph1_gens = {1: make_ph1(1)}
            fill(1)   # emit chunk-1 x prefetch DMAs now

            def pairburst(g):
                # emit up to two score pairs back to back, then fill
                if not step(g):
                    return False
                more = step(g)
                fillp()
                if more:
                    fillp()
                return more

            units = [(h, c) for c in range(NCHUNK) for h in range(HPC)]
            ets_map = {}
            sg_cur = {}
            qk_drained = {0: True}
            v_drained = {0: True}

            def start_sg(u):
                ets_map[u] = {}
                sg_cur[u] = scores_gen(u[0], u[1], ets_map[u])

            start_sg((0, 0))
            while pairburst(sg_cur[(0, 0)]):
                pass
            sg_cur[(0, 0)] = None

            po = {}
            for idx, (h, c) in enumerate(units):
                if h == 0:
                    if c + 1 < NCHUNK and c + 1 not in ph1_gens:
                        ph1_gens[c + 1] = make_ph1(c + 1)
                    if c >= 1 and not v_drained.get(c):
                        # pv of this chunk reads its v tiles: force any
                        # remaining v-chain emission first (in-order PE)
                        drain(ph1_gens[c][1])
                        v_drained[c] = True
                # lookahead: scores of the next 1-2 units interleave into
                # this unit's pv, keeping the scalar exp chain ahead
                acts = []
                for la in (1, 2):
                    if la == 2 and c == 0:
                        continue  # chunk-1 x lands too late at c=0
                    j2 = idx + la
                    if j2 >= len(units):
                        continue
                    w = units[j2]
                    wc = w[1]
                    if wc != 0 and not qk_drained.get(wc):
                        if wc not in ph1_gens:
                            ph1_gens[wc] = make_ph1(wc)
                        drain(ph1_gens[wc][0])
                        qk_drained[wc] = True
                    if w not in sg_cur:
                        start_sg(w)
                    if sg_cur.get(w):
                        acts.append(w)
                # current unit's scores must be fully emitted before its pv
                g = sg_cur.get((h, c))
                while g:
                    if not pairburst(g):
                        g = None
                sg_cur[(h, c)] = None
                pvg = pv_ln_gen(h, c, ets_map[(h, c)])
                quota = {w: (4 * (w[1] + 1) + 7) // 8 for w in acts}
                for j in range(4):
                    for w in acts:
                        gw = sg_cur.get(w)
                        n = 0
                        while gw and n < quota[w]:
                            if not pairburst(gw):
                                sg_cur[w] = None
                                gw = None
                            n += 1
                    step(pvg)
                    fill(FILLJ)
                drain(pvg)
                po[h] = tr(pv_ln_gen.out)
                if h == 1:
                    add_fill(proj_gen(c, [po[0], po[1]]), PROJ_EST)

            fill(1 << 30)

    from concourse import mybir as _mb
    _fix_sync_limits(nc, _mb, bass_rust)
    return nc


_NC_CACHE = {}


def _get_nc():
    if "nc" not in _NC_CACHE:
        _NC_CACHE["nc"] = _build_nc()
    return _NC_CACHE["nc"]


def kernel(x, W_qkv, W_o, lambda_q1, lambda_k1, lambda_q2, lambda_k2,
           gn_gamma, gn_beta):
    import os
    _install_ntff_shim()
    from concourse.bass_utils import run_bass_kernel_spmd

    x = np.asarray(x, np.float32)
    W_qkv = np.asarray(W_qkv, np.float32)
    W_o = np.asarray(W_o, np.float32)
    lambda_q1 = np.asarray(lambda_q1, np.float32)
    lambda_k1 = np.asarray(lambda_k1, np.float32)
    lambda_q2 = np.asarray(lambda_q2, np.float32)
    gn_gamma = np.asarray(gn_gamma, np.float32)
    gn_beta = np.asarray(gn_beta, np.float32)
    lambda_k2 = np.asarray(lambda_k2, np.float32)

    lambda_init = np.float32(0.8 - 0.6 * np.exp(-0.3 * LAYER_IDX))
    lam = (np.exp(lambda_q1 * lambda_k1) - np.exp(lambda_q2 * lambda_k2)
           + lambda_init).astype(np.float32)
    one_m_li = np.float32(1.0 - lambda_init)
    scale = np.float32(HEAD_DIM ** -0.5)

    import ml_dtypes
    x0T = np.ascontiguousarray(x[0].T).astype(ml_dtypes.bfloat16)
    # xR[c, p, d, col] = x0T[128d+p, 512c+col]
    xR = np.ascontiguousarray(
        x0T.reshape(NDT, 128, NCHUNK, CHUNK).transpose(2, 1, 0, 3))
    W3 = W_qkv.reshape(3, N_HEADS, 128, D_MODEL)
    tri = (np.arange(128)[None, :] >= np.arange(128)[:, None])  # [k, q]: k<=q
    tri2 = np.ascontiguousarray(
        np.concatenate([tri, tri], axis=1)).astype(ml_dtypes.bfloat16)

    in_maps = []
    for i in range(N_CORES):
        hs = [HPC * i + k for k in range(HPC)]
        wq = np.concatenate([W3[0, h] * scale for h in hs], 0)   # [256, D]
        wk = np.concatenate([W3[1, h] for h in hs], 0)           # [256, D]
        wv = np.concatenate([W3[2, h] for h in hs], 0)           # [256, D]
        wqkT_h = np.ascontiguousarray(
            np.concatenate([wq, wk], 0).T).astype(ml_dtypes.bfloat16)
        wvT_h = np.ascontiguousarray(wv.T).astype(ml_dtypes.bfloat16)
        # wqkR[p, d, col] = wqkT_h[128d+p, col]
        wqkR = np.ascontiguousarray(
            wqkT_h.reshape(NDT, 128, 512).transpose(1, 0, 2))
        wvR = np.ascontiguousarray(
            wvT_h.reshape(NDT, 128, HPC * 128).transpose(1, 0, 2))
        # gamma*(1-lambda_init) folded into W_o rows
        gfold = (gn_gamma[hs] * one_m_li).reshape(-1)            # [256]
        wo_cols = W_o[:, 128 * hs[0]:128 * (hs[-1] + 1)]         # [D, 256]
        woT_h = np.ascontiguousarray(
            (wo_cols * gfold[None, :]).T).astype(ml_dtypes.bfloat16)
        # -lam per value channel, broadcast to 128 partitions
        lamneg_bc = np.ascontiguousarray(
            np.broadcast_to(-lam[None, :], (128, 2 * HEAD_DIM)))
        lamneg_bc = np.concatenate([lamneg_bc] * HPC, axis=1).astype(np.float32)
        in_maps.append({
            "xT": xR,
            "wqkT": wqkR,
            "wvT": wvR,
            "woT": woT_h,
            "lamnegbc": np.ascontiguousarray(lamneg_bc),
            "tri2": tri2,
        })

    nc = _get_nc()
    trace = bool(int(os.environ.get("KERNEL_TRACE", "0")))
    res = run_bass_kernel_spmd(nc, in_maps, core_ids=list(range(N_CORES)),
                               trace=trace)
    if trace:
        _NC_CACHE["last_result"] = res
    yacc = np.zeros((SEQ, D_MODEL), np.float32)
    for r in res.results:
        yacc += np.asarray(r["y"], np.float32)
    # host-side rank-1 bias: sum_h W_o[:, h-block] @ (beta_h * (1-lam_init))
    bias = W_o @ (gn_beta.reshape(-1) * one_m_li)
    yacc += bias[None, :]
    return yacc[None]
